# revision 30
# baseline (speedup 1.0000x reference)
"""Sparse attention (per-query top-K) Trainium2 kernel, 8-core tensor-parallel.

Strategy (heads sharded 2-per-core, dense-score formulation, v2):
  - Host folds idx/valid/geo_bias into per-(s,q) merged bias factors
    E[s,q] = sum_{j: idx[q,j]==s} exp(geo_bias[h,q,j]), stored as causal
    fp16 tiles.  This turns the per-query gather/softmax into dense math:
        A^T = E^T * exp(S^T - C),   S^T = K @ Q^T (feature-major)
        out^T = [V | 1]^T @ A^T     (row 64 = softmax denominator)
  - x is pre-transposed on the host (xT bf16), so no DMA transposes.
  - Single software-pipelined loop: the projections for query-tile t+1
    (Q/K/V matmuls in bf16 with fp32 accumulation, V transposed on PE)
    are interleaved into the attention chunk loop of tile t, keeping the
    PE continuously busy (HAM stays un-throttled).
  - Per chunk: dense scores on PE (bf16, both heads concurrently via row
    groups), exp on ACT (both heads per instruction), E-multiply on DVE
    (fp16), AV on PE (fp16).  Diagonal chunks only compute the causal
    query range (qoff trimming).
  - After each tile, its [65,2,512] head-output slab is exchanged with a
    small per-tile AllToAll ([8,130,64] fp16: query-subblock j of tile t
    goes to core j), overlapped with the next tile's compute.  Core c
    ends up owning query rows t*512 + c*64 + [0,64) for all t; the host
    reassembles.  Phase 3 (normalize + o_proj row-slice in bf16) runs on
    data prefetched as the collectives complete.
"""

import sys

sys.path.insert(0, "/opt/trn_rl_repo")

import numpy as np
import ml_dtypes

from concourse import bacc, mybir, tile
from concourse.bass_utils import run_bass_kernel_spmd
from concourse.masks import make_identity

F32 = mybir.dt.float32
F32R = mybir.dt.float32r
F16 = mybir.dt.float16
BF16 = mybir.dt.bfloat16

S = 4096
H = 1024
NH = 16
KSEL = 32
HD = 64
NC = 8
HPC = NH // NC  # 2 heads per core
QT = 512
NQT = S // QT
SC = 128
CSHIFT = 2.0
SLAB = 4  # s-chunks per E-tile DMA slab

# Per-tile chunk iteration order: diagonal chunks first (ascending, so the
# first AV matmul covers the full query range), then off-diagonal chunks.
CHUNKS = {t: list(range(4 * t, 4 * t + 4)) + list(range(0, 4 * t)) for t in range(NQT)}
TILE_LIST = [(t, c) for t in range(NQT) for c in CHUNKS[t]]
N_TILES = len(TILE_LIST)  # 144


def _qoff(t, c):
    # query-range trim for diagonal chunks (keys all above queries < qoff)
    if t == 0 or c < 4 * t:
        return 0
    return (c - 4 * t) * SC


def _build_program(n_cores_build=NC):
    nc = bacc.Bacc(
        "TRN2", target_bir_lowering=False, debug=False, num_devices=n_cores_build
    )

    xt_in = nc.dram_tensor("xt", [H, S], BF16, kind="ExternalInput").ap()
    wq_in = nc.dram_tensor("wq", [H, 128], BF16, kind="ExternalInput").ap()
    wk_in = nc.dram_tensor("wk", [H, 128], BF16, kind="ExternalInput").ap()
    wv_in = nc.dram_tensor("wv", [H, 128], BF16, kind="ExternalInput").ap()
    wo_in = nc.dram_tensor("wo", [H, H], BF16, kind="ExternalInput").ap()
    bo_in = nc.dram_tensor("bo_rep", [128, H], F32, kind="ExternalInput").ap()
    e_in = nc.dram_tensor(
        "e_pack", [N_TILES, SC, HPC, QT], F16, kind="ExternalInput"
    ).ap()
    sel_in = nc.dram_tensor("sel16", [NH, H], F32R, kind="ExternalInput").ap()
    y_out = nc.dram_tensor("y_part", [QT, H], F32, kind="ExternalOutput").ap()

    with tile.TileContext(nc) as tc:
        with (
            tc.tile_pool(name="const", bufs=1) as constp,
            tc.tile_pool(name="persist", bufs=1) as persist,
            tc.tile_pool(name="dram", bufs=1, space="DRAM") as dram,
            tc.tile_pool(name="xT", bufs=2) as xTp,
            tc.tile_pool(name="vtmp", bufs=2) as vtmpp,
            tc.tile_pool(name="ep", bufs=8) as epool,
            tc.tile_pool(name="zap", bufs=6) as zap,
            tc.tile_pool(name="otp", bufs=2) as otp,
            tc.tile_pool(name="p3y", bufs=2) as p3y,
        ):
            ident_bf = constp.tile([128, 128], BF16, tag="identb")
            make_identity(nc, ident_bf[:])
            nbias = constp.tile([128, 1], F32, tag="nbias")
            nc.gpsimd.memset(nbias[:], -CSHIFT)

            wq_sb = constp.tile([128, 8, 128], BF16, tag="wq")
            wk_sb = constp.tile([128, 8, 128], BF16, tag="wk")
            wv_sb = constp.tile([128, 8, 128], BF16, tag="wv")
            nc.sync.dma_start(wq_sb[:], wq_in.rearrange("(c p) m -> p c m", p=128))
            nc.sync.dma_start(wk_sb[:], wk_in.rearrange("(c p) m -> p c m", p=128))
            nc.sync.dma_start(wv_sb[:], wv_in.rearrange("(c p) m -> p c m", p=128))
            wo_sb = constp.tile([128, 8, H], BF16, tag="wo")
            nc.sync.dma_start(wo_sb[:], wo_in.rearrange("(c p) f -> p c f", p=128))
            bo_sb = constp.tile([128, H], F32, tag="bo")
            nc.sync.dma_start(bo_sb[:], bo_in[:])
            sel_sb = constp.tile([NH, H], F32R, tag="sel")
            nc.sync.dma_start(sel_sb[:], sel_in[:])

            qT_sb = persist.tile([128, NQT, QT], BF16, tag="qT")
            kT_sb = persist.tile([128, NQT, QT], BF16, tag="kT")
            v_sb = [
                persist.tile([128, S // SC, HD + 1], F16, tag=f"v{h}", name=f"v{h}")
                for h in range(HPC)
            ]
            for h in range(HPC):
                nc.gpsimd.memset(v_sb[h][:], 1.0)

            den_sb = persist.tile([NH, QT], F16, tag="den")
            oT_sb = persist.tile([128, 8, QT], F16, tag="oT")
            rden_sb = persist.tile([NH, QT], F32, tag="rden")
            rden_r = persist.tile([NH, QT], F32R, tag="rdenr")
            on_sb = persist.tile([128, 8, QT], BF16, tag="on")

            # collectives batched per tile-group: [dest core, 2*65 rows, tile u, 64]
            GROUPS = [[0, 1], [2, 3], [4, 5], [6], [7]]
            GRP_OF = {t: gi for gi, g in enumerate(GROUPS) for t in g}
            a2a_in = [
                dram.tile([NC, HPC * (HD + 1), len(g), 64], F16, name=f"a2a_in{gi}")
                for gi, g in enumerate(GROUPS)
            ]
            a2a_out = [
                dram.tile([NC, HPC * (HD + 1), len(g), 64], F16, name=f"a2a_out{gi}")
                for gi, g in enumerate(GROUPS)
            ]

            xt_view = xt_in.rearrange("(c p) s -> p c s", p=128)
            XT = {}

            def load_xt(t):
                XT[t] = xTp.tile([128, 8, QT], BF16, tag="xt", name="xT_t")
                nc.sync.dma_start(XT[t][:], xt_view[:, :, t * QT : (t + 1) * QT])

            def p3_fetch(t):
                # fetch den/oT columns for tile t from its landed a2a
                gi = GRP_OF[t]
                u = t - GROUPS[gi][0]
                for l in range(HPC):
                    nc.gpsimd.dma_start(
                        oT_sb[l * HD : (l + 1) * HD, :, t * 64 : (t + 1) * 64],
                        a2a_out[gi][:, l * 65 : l * 65 + HD, u, :].rearrange(
                            "c d j -> d c j"
                        ),
                    )
                    nc.gpsimd.dma_start(
                        den_sb[l * 8 : (l + 1) * 8, t * 64 : (t + 1) * 64],
                        a2a_out[gi][:, l * 65 + HD, u, :],
                    )

            with (
                tc.tile_pool(name="pp", bufs=2, space="PSUM") as pp,
                tc.tile_pool(name="p2s", bufs=2, space="PSUM") as p2s,
                tc.tile_pool(name="p2o", bufs=1, space="PSUM") as p2o,
            ):

                def emit_projection(t):
                    """Thunk list computing q/k/v(+transpose) for s-tile t."""
                    ops = []
                    xT_t = XT[t]
                    vT_tmp = vtmpp.tile([128, QT], BF16, tag="vt")
                    for kind in ("q", "k", "v"):
                        w_sb = {"q": wq_sb, "k": wk_sb, "v": wv_sb}[kind]
                        ps = pp.tile([128, QT], F32, tag="pj", name=f"ps_{kind}")
                        for c in range(8):
                            ops.append(
                                lambda ps=ps, w_sb=w_sb, c=c: nc.tensor.matmul(
                                    ps[:], w_sb[:, c, :], xT_t[:, c, :],
                                    start=(c == 0), stop=(c == 7),
                                )
                            )
                        if kind == "q":
                            ops.append(
                                lambda ps=ps: nc.vector.tensor_copy(
                                    qT_sb[:, t, :], ps[:]
                                )
                            )
                        elif kind == "k":
                            ops.append(
                                lambda ps=ps: nc.scalar.copy(kT_sb[:, t, :], ps[:])
                            )
                        else:
                            ops.append(lambda ps=ps: nc.scalar.copy(vT_tmp[:], ps[:]))
                    ps_tv = pp.tile([128, QT], BF16, tag="pj", name="ps_tv")
                    for i in range(4):
                        ops.append(
                            lambda i=i: nc.tensor.transpose(
                                ps_tv[:, i * 128 : (i + 1) * 128],
                                vT_tmp[:, i * 128 : (i + 1) * 128],
                                ident_bf[:],
                            )
                        )

                    def copy_v():
                        ps_tv4 = ps_tv[:].rearrange("p (i h d) -> p i h d", i=4, h=HPC)
                        for h in range(HPC):
                            nc.vector.tensor_copy(
                                v_sb[h][:, t * 4 : (t + 1) * 4, 0:HD],
                                ps_tv4[:, :, h, :],
                            )

                    ops.append(copy_v)
                    return ops

                def emit_p3_block(gi):
                    """Thunks: fetch + normalize + o_proj for tile-group gi
                    (output rows g[0]*64 .. (g[-1]+1)*64)."""
                    g = GROUPS[gi]
                    W = 64 * len(g)
                    c0 = g[0] * 64
                    ops = []
                    cols = slice(c0, c0 + W)
                    for tt in g:
                        ops.append(lambda tt=tt: p3_fetch(tt))
                    ops.append(
                        lambda: nc.vector.reciprocal(
                            rden_sb[:, cols], den_sb[:, cols]
                        )
                    )
                    ops.append(
                        lambda: nc.vector.tensor_copy(
                            rden_r[:, cols], rden_sb[:, cols]
                        )
                    )
                    for ci in range(8):
                        ps_b = pp.tile([128, QT], F32, tag="pj", name="ps_b")
                        ops.append(
                            lambda ps_b=ps_b, ci=ci: nc.tensor.matmul(
                                ps_b[:, 0:W],
                                sel_sb[:, ci * 128 : (ci + 1) * 128],
                                rden_r[:, cols],
                                start=True,
                                stop=True,
                            )
                        )
                        ops.append(
                            lambda ps_b=ps_b, ci=ci: nc.vector.tensor_mul(
                                on_sb[:, ci, cols],
                                oT_sb[:, ci, cols],
                                ps_b[:, 0:W],
                            )
                        )
                    y_sb = p3y.tile([128, H], F32, tag="y")
                    for nh2 in range(2):
                        ps_y = pp.tile([128, QT], F32, tag="pj", name="ps_y")
                        for c in range(8):
                            ops.append(
                                lambda ps_y=ps_y, c=c, nh2=nh2: nc.tensor.matmul(
                                    ps_y[0:W, :],
                                    on_sb[:, c, cols],
                                    wo_sb[:, c, nh2 * QT : (nh2 + 1) * QT],
                                    start=(c == 0),
                                    stop=(c == 7),
                                )
                            )
                        ops.append(
                            lambda ps_y=ps_y, nh2=nh2: nc.vector.tensor_add(
                                y_sb[0:W, nh2 * QT : (nh2 + 1) * QT],
                                ps_y[0:W, :],
                                bo_sb[0:W, nh2 * QT : (nh2 + 1) * QT],
                            )
                        )
                    ops.append(
                        lambda: nc.sync.dma_start(
                            y_out[c0 : c0 + W, :], y_sb[0:W, :]
                        )
                    )
                    return ops

                load_xt(0)
                load_xt(1)
                for op in emit_projection(0):
                    op()

                pending = []
                tile_pos = 0  # index into TILE_LIST
                for t in range(NQT):
                    if t + 2 < NQT:
                        load_xt(t + 2)
                    pending = emit_projection(t + 1) if t + 1 < NQT else []
                    # phase-3 blocks scheduled once their collective is safely
                    # landed; group 3 ({6}) overlaps the tail of tile 7
                    BLOCK_AT = {4: [0], 6: [1], 7: [2, 3]}
                    for gi in BLOCK_AT.get(t, []):
                        pending = pending + emit_p3_block(gi)

                    chunks = CHUNKS[t]
                    nchunks = len(chunks)
                    slabs = []
                    for g0 in range(0, nchunks, SLAB):
                        gsz = min(SLAB, nchunks - g0)
                        e_slab = epool.tile(
                            [128, SLAB, HPC, QT], F16, tag="e", name="e_slab"
                        )
                        n0 = tile_pos + g0
                        src = e_in[n0 : n0 + gsz].rearrange("n p h q -> p n h q")
                        nc.sync.dma_start(e_slab[:, 0:gsz, :, :], src)
                        slabs.append(e_slab)
                    tile_pos += nchunks

                    ps_o = [
                        p2o.tile([HD + 1, QT], F32, tag=f"po{h}", name=f"po{h}")
                        for h in range(HPC)
                    ]

                    def emit_scores(ci, c):
                        qo = _qoff(t, c)
                        ps_s2 = p2s.tile([128, HPC, QT], F32, tag="ps2", name="ps_s2")
                        for h in range(HPC):
                            nc.tensor.matmul(
                                ps_s2[:, h, qo:QT],
                                kT_sb[
                                    h * HD : (h + 1) * HD,
                                    c // 4,
                                    (c % 4) * 128 : (c % 4 + 1) * 128,
                                ],
                                qT_sb[h * HD : (h + 1) * HD, t, qo:QT],
                                start=True,
                                stop=True,
                            )
                        return ps_s2

                    score_ps = emit_scores(0, chunks[0])
                    n_emitted = 0
                    for ci, c in enumerate(chunks):
                        e_slab = slabs[ci // SLAB]
                        c_loc = ci % SLAB
                        qo = _qoff(t, c)
                        ps_s2 = score_ps
                        if ci + 1 < nchunks:
                            score_ps = emit_scores(ci + 1, chunks[ci + 1])
                        # drip-feed next tile's projection into engine queues
                        # (before the AV matmuls so the PE can fill the
                        # exp/mult latency with useful work)
                        target = ((ci + 1) * len(pending) + nchunks - 1) // nchunks
                        while n_emitted < min(target, len(pending)):
                            pending[n_emitted]()
                            n_emitted += 1
                        z_sb = zap.tile([128, HPC, QT], F16, tag="z")
                        nc.scalar.activation(
                            z_sb[:, :, qo:QT],
                            ps_s2[:, :, qo:QT],
                            mybir.ActivationFunctionType.Exp,
                            bias=nbias[:],
                        )
                        a_sb = zap.tile([128, HPC, QT], F16, tag="a")
                        nc.vector.tensor_mul(
                            a_sb[:, :, qo:QT],
                            z_sb[:, :, qo:QT],
                            e_slab[:, c_loc, :, qo:QT],
                        )
                        for h in range(HPC):
                            nc.tensor.matmul(
                                ps_o[h][:, qo:QT],
                                v_sb[h][:, c, :],
                                a_sb[:, h, qo:QT],
                                start=(ci == 0),
                                stop=(ci == nchunks - 1),
                            )
                    while n_emitted < len(pending):
                        pending[n_emitted]()
                        n_emitted += 1

                    ot_sb = otp.tile([HD + 1, HPC, QT], F16, tag="ot")
                    for h in range(HPC):
                        nc.vector.tensor_copy(ot_sb[:, h, :], ps_o[h][:])
                    gi = GRP_OF[t]
                    u = t - GROUPS[gi][0]
                    for l in range(HPC):
                        nc.sync.dma_start(
                            a2a_in[gi][:, l * 65 : (l + 1) * 65, u, :].rearrange(
                                "p d j -> d p j"
                            ),
                            ot_sb[:, l, :].rearrange("d (p j) -> d p j", p=NC),
                        )
                    if t == GROUPS[gi][-1]:
                        nc.gpsimd.collective_compute(
                            "AllToAll",
                            mybir.AluOpType.bypass,
                            replica_groups=[list(range(NC))],
                            ins=[a2a_in[gi].opt()],
                            outs=[a2a_out[gi].opt()],
                        )

                # remaining phase-3 block (group 4 = tile 7)
                for op in emit_p3_block(4):
                    op()

    nc.compile()
    return nc


_PROGRAM_CACHE = {}


def _get_program():
    if "nc" not in _PROGRAM_CACHE:
        _PROGRAM_CACHE["nc"] = _build_program()
    return _PROGRAM_CACHE["nc"]


def _host_prep(x, idx, valid, geo_bias, Wq, Wk, Wv, Wo, bo):
    x2 = np.ascontiguousarray(np.asarray(x, dtype=np.float32).reshape(S, H))
    idx = np.asarray(idx).astype(np.int64)
    valid = np.asarray(valid).astype(bool)
    geo = np.asarray(geo_bias, dtype=np.float32)
    Wq = np.asarray(Wq, dtype=np.float32)
    Wk = np.asarray(Wk, dtype=np.float32)
    Wv = np.asarray(Wv, dtype=np.float32)
    Wo = np.asarray(Wo, dtype=np.float32)
    bo = np.asarray(bo, dtype=np.float32)

    qpos = np.arange(S, dtype=np.int64)[:, None]
    keep = valid & (idx <= qpos) & (idx >= 0)
    s_flat = idx[keep]
    q_flat = np.broadcast_to(qpos, idx.shape)[keep]
    lin = s_flat * S + q_flat

    bo_rep = np.ascontiguousarray(np.broadcast_to(bo[None, :], (128, H)))

    # den row order in phase 3 is r = l*8 + ci for head h = 2*ci + l
    sel16 = np.zeros((NH, H), dtype=np.float32)
    ch = np.arange(H)
    sel16[((ch // HD) % 2) * 8 + ch // 128, ch] = 1.0

    wq_scaled = Wq / np.sqrt(HD)
    xt_bf = np.ascontiguousarray(x2.T).astype(ml_dtypes.bfloat16)

    in_maps = []
    for core in range(NC):
        e_pack = np.empty((N_TILES, SC, HPC, QT), dtype=np.float16)
        for l in range(HPC):
            h = HPC * core + l
            w = np.exp(geo[h][keep].astype(np.float64))
            eT = np.bincount(lin, weights=w, minlength=S * S).reshape(S, S)
            for n, (t, c) in enumerate(TILE_LIST):
                e_pack[n, :, l, :] = eT[
                    c * SC : (c + 1) * SC, t * QT : (t + 1) * QT
                ].astype(np.float16)
        cs = slice(128 * core, 128 * (core + 1))
        in_maps.append(
            {
                "xt": xt_bf,
                "wq": np.ascontiguousarray(wq_scaled[:, cs]).astype(ml_dtypes.bfloat16),
                "wk": np.ascontiguousarray(Wk[:, cs]).astype(ml_dtypes.bfloat16),
                "wv": np.ascontiguousarray(Wv[:, cs]).astype(ml_dtypes.bfloat16),
                "wo": Wo.astype(ml_dtypes.bfloat16),
                "bo_rep": bo_rep,
                "e_pack": e_pack,
                "sel16": sel16,
            }
        )
    return in_maps


def kernel(x, idx, valid, geo_bias, Wq, Wk, Wv, Wo, bo):
    b, s, h = np.asarray(x).shape
    assert (b, s, h) == (1, S, H)
    in_maps = _host_prep(x, idx, valid, geo_bias, Wq, Wk, Wv, Wo, bo)
    nc = _get_program()
    res = run_bass_kernel_spmd(nc, in_maps, core_ids=list(range(NC)))
    # core c owns query rows t*512 + c*64 + j at its local column t*64 + j
    parts = np.stack([res.results[c]["y_part"] for c in range(NC)])  # [c, 512, H]
    y = parts.reshape(NC, NQT, 64, H).transpose(1, 0, 2, 3).reshape(S, H)
    return y.reshape(1, S, H).astype(np.float32)


# revision 31
# speedup vs baseline: 1.1721x; 1.1721x over previous
"""Sparse attention (per-query top-K) Trainium2 kernel, 8-core tensor-parallel.

Strategy (heads sharded 2-per-core, dense-score formulation, v2):
  - Host folds idx/valid/geo_bias into per-(s,q) merged bias factors
    E[s,q] = sum_{j: idx[q,j]==s} exp(geo_bias[h,q,j]), stored as causal
    fp16 tiles.  This turns the per-query gather/softmax into dense math:
        A^T = E^T * exp(S^T - C),   S^T = K @ Q^T (feature-major)
        out^T = [V | 1]^T @ A^T     (row 64 = softmax denominator)
  - x is pre-transposed on the host (xT bf16), so no DMA transposes.
  - Single software-pipelined loop: the projections for query-tile t+1
    (Q/K/V matmuls in bf16 with fp32 accumulation, V transposed on PE)
    are interleaved into the attention chunk loop of tile t, keeping the
    PE continuously busy (HAM stays un-throttled).
  - Per chunk: dense scores on PE (bf16, both heads concurrently via row
    groups), exp on ACT (both heads per instruction), E-multiply on DVE
    (fp16), AV on PE (fp16).  Diagonal chunks only compute the causal
    query range (qoff trimming).
  - After each tile, its [65,2,512] head-output slab is exchanged with a
    small per-tile AllToAll ([8,130,64] fp16: query-subblock j of tile t
    goes to core j), overlapped with the next tile's compute.  Core c
    ends up owning query rows t*512 + c*64 + [0,64) for all t; the host
    reassembles.  Phase 3 (normalize + o_proj row-slice in bf16) runs on
    data prefetched as the collectives complete.
"""

import sys

sys.path.insert(0, "/opt/trn_rl_repo")

import numpy as np
import ml_dtypes

from concourse import bacc, mybir, tile
from concourse.bass_utils import run_bass_kernel_spmd
from concourse.masks import make_identity

F32 = mybir.dt.float32
F32R = mybir.dt.float32r
F16 = mybir.dt.float16
BF16 = mybir.dt.bfloat16

S = 4096
H = 1024
NH = 16
KSEL = 32
HD = 64
NC = 8
HPC = NH // NC  # 2 heads per core
QT = 512
NQT = S // QT
SC = 128
CSHIFT = 2.0
SLAB = 4  # s-chunks per E-tile DMA slab

# Per-tile chunk iteration order: diagonal chunks first (ascending, so the
# first AV matmul covers the full query range), then off-diagonal chunks.
CHUNKS = {t: list(range(4 * t, 4 * t + 4)) + list(range(0, 4 * t)) for t in range(NQT)}
TILE_LIST = [(t, c) for t in range(NQT) for c in CHUNKS[t]]
N_TILES = len(TILE_LIST)  # 144


def _qoff(t, c):
    # query-range trim for diagonal chunks (keys all above queries < qoff)
    if t == 0 or c < 4 * t:
        return 0
    return (c - 4 * t) * SC


def _build_program(n_cores_build=NC):
    nc = bacc.Bacc(
        "TRN2", target_bir_lowering=False, debug=False, num_devices=n_cores_build
    )

    xt_in = nc.dram_tensor("xt", [H, S], BF16, kind="ExternalInput").ap()
    wq_in = nc.dram_tensor("wq", [H, 128], BF16, kind="ExternalInput").ap()
    wk_in = nc.dram_tensor("wk", [H, 128], BF16, kind="ExternalInput").ap()
    wv_in = nc.dram_tensor("wv", [H, 128], BF16, kind="ExternalInput").ap()
    wo_in = nc.dram_tensor("wo", [H, H], BF16, kind="ExternalInput").ap()
    bo_in = nc.dram_tensor("bo_rep", [128, H], F32, kind="ExternalInput").ap()
    e_in = nc.dram_tensor(
        "e_pack", [N_TILES, SC, HPC, QT], F16, kind="ExternalInput"
    ).ap()
    sel_in = nc.dram_tensor("sel16", [NH, H], F32R, kind="ExternalInput").ap()
    y_out = nc.dram_tensor("y_part", [QT, H], F32, kind="ExternalOutput").ap()

    with tile.TileContext(nc) as tc:
        with (
            tc.tile_pool(name="const", bufs=1) as constp,
            tc.tile_pool(name="persist", bufs=1) as persist,
            tc.tile_pool(name="dram", bufs=1, space="DRAM") as dram,
            tc.tile_pool(name="xT", bufs=2) as xTp,
            tc.tile_pool(name="vtmp", bufs=2) as vtmpp,
            tc.tile_pool(name="ep", bufs=8) as epool,
            tc.tile_pool(name="zap", bufs=6) as zap,
            tc.tile_pool(name="otp", bufs=2) as otp,
            tc.tile_pool(name="p3y", bufs=2) as p3y,
        ):
            ident_bf = constp.tile([128, 128], BF16, tag="identb")
            make_identity(nc, ident_bf[:])
            nbias = constp.tile([128, 1], F32, tag="nbias")
            nc.gpsimd.memset(nbias[:], -CSHIFT)

            wq_sb = constp.tile([128, 8, 128], BF16, tag="wq")
            wk_sb = constp.tile([128, 8, 128], BF16, tag="wk")
            wv_sb = constp.tile([128, 8, 128], BF16, tag="wv")
            nc.sync.dma_start(wq_sb[:], wq_in.rearrange("(c p) m -> p c m", p=128))
            nc.sync.dma_start(wk_sb[:], wk_in.rearrange("(c p) m -> p c m", p=128))
            nc.sync.dma_start(wv_sb[:], wv_in.rearrange("(c p) m -> p c m", p=128))
            wo_sb = constp.tile([128, 8, H], BF16, tag="wo")
            nc.sync.dma_start(wo_sb[:], wo_in.rearrange("(c p) f -> p c f", p=128))
            bo_sb = constp.tile([128, H], F32, tag="bo")
            nc.sync.dma_start(bo_sb[:], bo_in[:])
            sel_sb = constp.tile([NH, H], F32R, tag="sel")
            nc.sync.dma_start(sel_sb[:], sel_in[:])

            qT_sb = persist.tile([128, NQT, QT], BF16, tag="qT")
            kT_sb = persist.tile([128, NQT, QT], BF16, tag="kT")
            v_sb = [
                persist.tile([128, S // SC, HD + 1], F16, tag=f"v{h}", name=f"v{h}")
                for h in range(HPC)
            ]
            for h in range(HPC):
                nc.gpsimd.memset(v_sb[h][:], 1.0)

            den_sb = persist.tile([NH, QT], F16, tag="den")
            oT_sb = persist.tile([128, 8, QT], F16, tag="oT")
            rden_sb = persist.tile([NH, QT], F32, tag="rden")
            rden_r = persist.tile([NH, QT], F32R, tag="rdenr")
            on_sb = persist.tile([128, 8, QT], BF16, tag="on")

            # one collective per tile: [dest core, 2*65 rows, tile u, 64]
            GROUPS = [[t] for t in range(NQT)]
            GRP_OF = {t: gi for gi, g in enumerate(GROUPS) for t in g}
            a2a_in = [
                dram.tile([NC, HPC * (HD + 1), len(g), 64], F16, name=f"a2a_in{gi}")
                for gi, g in enumerate(GROUPS)
            ]
            a2a_out = [
                dram.tile([NC, HPC * (HD + 1), len(g), 64], F16, name=f"a2a_out{gi}")
                for gi, g in enumerate(GROUPS)
            ]

            xt_view = xt_in.rearrange("(c p) s -> p c s", p=128)
            XT = {}

            def load_xt(t):
                XT[t] = xTp.tile([128, 8, QT], BF16, tag="xt", name="xT_t")
                nc.sync.dma_start(XT[t][:], xt_view[:, :, t * QT : (t + 1) * QT])

            def p3_fetch(t):
                # fetch den/oT columns for tile t from its landed a2a
                gi = GRP_OF[t]
                u = t - GROUPS[gi][0]
                for l in range(HPC):
                    nc.gpsimd.dma_start(
                        oT_sb[l * HD : (l + 1) * HD, :, t * 64 : (t + 1) * 64],
                        a2a_out[gi][:, l * 65 : l * 65 + HD, u, :].rearrange(
                            "c d j -> d c j"
                        ),
                    )
                    nc.gpsimd.dma_start(
                        den_sb[l * 8 : (l + 1) * 8, t * 64 : (t + 1) * 64],
                        a2a_out[gi][:, l * 65 + HD, u, :],
                    )

            with (
                tc.tile_pool(name="pp", bufs=2, space="PSUM") as pp,
                tc.tile_pool(name="p2s", bufs=2, space="PSUM") as p2s,
                tc.tile_pool(name="p2o", bufs=1, space="PSUM") as p2o,
            ):

                def emit_projection(t):
                    """Thunk list computing q/k/v(+transpose) for s-tile t."""
                    ops = []
                    xT_t = XT[t]
                    vT_tmp = vtmpp.tile([128, QT], BF16, tag="vt")
                    for kind in ("q", "k", "v"):
                        w_sb = {"q": wq_sb, "k": wk_sb, "v": wv_sb}[kind]
                        ps = pp.tile([128, QT], F32, tag="pj", name=f"ps_{kind}")
                        for c in range(8):
                            ops.append(
                                lambda ps=ps, w_sb=w_sb, c=c: nc.tensor.matmul(
                                    ps[:], w_sb[:, c, :], xT_t[:, c, :],
                                    start=(c == 0), stop=(c == 7),
                                )
                            )
                        if kind == "q":
                            ops.append(
                                lambda ps=ps: nc.vector.tensor_copy(
                                    qT_sb[:, t, :], ps[:]
                                )
                            )
                        elif kind == "k":
                            ops.append(
                                lambda ps=ps: nc.scalar.copy(kT_sb[:, t, :], ps[:])
                            )
                        else:
                            ops.append(lambda ps=ps: nc.scalar.copy(vT_tmp[:], ps[:]))
                    ps_tv = pp.tile([128, QT], BF16, tag="pj", name="ps_tv")
                    for i in range(4):
                        ops.append(
                            lambda i=i: nc.tensor.transpose(
                                ps_tv[:, i * 128 : (i + 1) * 128],
                                vT_tmp[:, i * 128 : (i + 1) * 128],
                                ident_bf[:],
                            )
                        )

                    def copy_v():
                        ps_tv4 = ps_tv[:].rearrange("p (i h d) -> p i h d", i=4, h=HPC)
                        for h in range(HPC):
                            nc.vector.tensor_copy(
                                v_sb[h][:, t * 4 : (t + 1) * 4, 0:HD],
                                ps_tv4[:, :, h, :],
                            )

                    ops.append(copy_v)
                    return ops

                def emit_p3_block(gi):
                    """Thunks: fetch + normalize + o_proj for tile-group gi
                    (output rows g[0]*64 .. (g[-1]+1)*64)."""
                    g = GROUPS[gi]
                    W = 64 * len(g)
                    c0 = g[0] * 64
                    ops = []
                    cols = slice(c0, c0 + W)
                    for tt in g:
                        ops.append(lambda tt=tt: p3_fetch(tt))
                    ops.append(
                        lambda: nc.vector.reciprocal(
                            rden_sb[:, cols], den_sb[:, cols]
                        )
                    )
                    ops.append(
                        lambda: nc.vector.tensor_copy(
                            rden_r[:, cols], rden_sb[:, cols]
                        )
                    )
                    for ci in range(8):
                        ps_b = pp.tile([128, QT], F32, tag="pj", name="ps_b")
                        ops.append(
                            lambda ps_b=ps_b, ci=ci: nc.tensor.matmul(
                                ps_b[:, 0:W],
                                sel_sb[:, ci * 128 : (ci + 1) * 128],
                                rden_r[:, cols],
                                start=True,
                                stop=True,
                            )
                        )
                        ops.append(
                            lambda ps_b=ps_b, ci=ci: nc.vector.tensor_mul(
                                on_sb[:, ci, cols],
                                oT_sb[:, ci, cols],
                                ps_b[:, 0:W],
                            )
                        )
                    y_sb = p3y.tile([128, H], F32, tag="y")
                    for nh2 in range(2):
                        ps_y = pp.tile([128, QT], F32, tag="pj", name="ps_y")
                        for c in range(8):
                            ops.append(
                                lambda ps_y=ps_y, c=c, nh2=nh2: nc.tensor.matmul(
                                    ps_y[0:W, :],
                                    on_sb[:, c, cols],
                                    wo_sb[:, c, nh2 * QT : (nh2 + 1) * QT],
                                    start=(c == 0),
                                    stop=(c == 7),
                                )
                            )
                        ops.append(
                            lambda ps_y=ps_y, nh2=nh2: nc.vector.tensor_add(
                                y_sb[0:W, nh2 * QT : (nh2 + 1) * QT],
                                ps_y[0:W, :],
                                bo_sb[0:W, nh2 * QT : (nh2 + 1) * QT],
                            )
                        )
                    ops.append(
                        lambda: nc.sync.dma_start(
                            y_out[c0 : c0 + W, :], y_sb[0:W, :]
                        )
                    )
                    return ops

                load_xt(0)
                load_xt(1)
                for op in emit_projection(0):
                    op()

                pending = []
                tile_pos = 0  # index into TILE_LIST
                for t in range(NQT):
                    if t + 2 < NQT:
                        load_xt(t + 2)
                    pending = emit_projection(t + 1) if t + 1 < NQT else []
                    # phase-3 blocks scheduled once their collective is safely
                    # landed; group 3 ({6}) overlaps the tail of tile 7
                    BLOCK_AT = {4: [0], 6: [1], 7: [2, 3]}
                    for gi in BLOCK_AT.get(t, []):
                        pending = pending + emit_p3_block(gi)

                    chunks = CHUNKS[t]
                    nchunks = len(chunks)
                    slabs = []
                    for g0 in range(0, nchunks, SLAB):
                        gsz = min(SLAB, nchunks - g0)
                        e_slab = epool.tile(
                            [128, SLAB, HPC, QT], F16, tag="e", name="e_slab"
                        )
                        n0 = tile_pos + g0
                        src = e_in[n0 : n0 + gsz].rearrange("n p h q -> p n h q")
                        nc.sync.dma_start(e_slab[:, 0:gsz, :, :], src)
                        slabs.append(e_slab)
                    tile_pos += nchunks

                    ps_o = [
                        p2o.tile([HD + 1, QT], F32, tag=f"po{h}", name=f"po{h}")
                        for h in range(HPC)
                    ]

                    def emit_scores(ci, c):
                        qo = _qoff(t, c)
                        ps_s2 = p2s.tile([128, HPC, QT], F32, tag="ps2", name="ps_s2")
                        for h in range(HPC):
                            nc.tensor.matmul(
                                ps_s2[:, h, qo:QT],
                                kT_sb[
                                    h * HD : (h + 1) * HD,
                                    c // 4,
                                    (c % 4) * 128 : (c % 4 + 1) * 128,
                                ],
                                qT_sb[h * HD : (h + 1) * HD, t, qo:QT],
                                start=True,
                                stop=True,
                            )
                        return ps_s2

                    score_ps = emit_scores(0, chunks[0])
                    n_emitted = 0
                    for ci, c in enumerate(chunks):
                        e_slab = slabs[ci // SLAB]
                        c_loc = ci % SLAB
                        qo = _qoff(t, c)
                        ps_s2 = score_ps
                        if ci + 1 < nchunks:
                            score_ps = emit_scores(ci + 1, chunks[ci + 1])
                        # drip-feed next tile's projection into engine queues
                        # (before the AV matmuls so the PE can fill the
                        # exp/mult latency with useful work)
                        target = ((ci + 1) * len(pending) + nchunks - 1) // nchunks
                        while n_emitted < min(target, len(pending)):
                            pending[n_emitted]()
                            n_emitted += 1
                        z_sb = zap.tile([128, HPC, QT], F16, tag="z")
                        nc.scalar.activation(
                            z_sb[:, :, qo:QT],
                            ps_s2[:, :, qo:QT],
                            mybir.ActivationFunctionType.Exp,
                            bias=nbias[:],
                        )
                        a_sb = zap.tile([128, HPC, QT], F16, tag="a")
                        nc.vector.tensor_mul(
                            a_sb[:, :, qo:QT],
                            z_sb[:, :, qo:QT],
                            e_slab[:, c_loc, :, qo:QT],
                        )
                        for h in range(HPC):
                            nc.tensor.matmul(
                                ps_o[h][:, qo:QT],
                                v_sb[h][:, c, :],
                                a_sb[:, h, qo:QT],
                                start=(ci == 0),
                                stop=(ci == nchunks - 1),
                            )
                    while n_emitted < len(pending):
                        pending[n_emitted]()
                        n_emitted += 1

                    ot_sb = otp.tile([HD + 1, HPC, QT], F16, tag="ot")
                    for h in range(HPC):
                        nc.vector.tensor_copy(ot_sb[:, h, :], ps_o[h][:])
                    gi = GRP_OF[t]
                    u = t - GROUPS[gi][0]
                    for l in range(HPC):
                        nc.sync.dma_start(
                            a2a_in[gi][:, l * 65 : (l + 1) * 65, u, :].rearrange(
                                "p d j -> d p j"
                            ),
                            ot_sb[:, l, :].rearrange("d (p j) -> d p j", p=NC),
                        )
                    if t == GROUPS[gi][-1]:
                        nc.gpsimd.collective_compute(
                            "AllToAll",
                            mybir.AluOpType.bypass,
                            replica_groups=[list(range(NC))],
                            ins=[a2a_in[gi].opt()],
                            outs=[a2a_out[gi].opt()],
                        )

                # remaining phase-3 block (group 4 = tile 7)
                for op in emit_p3_block(4):
                    op()

    nc.compile()
    return nc


_PROGRAM_CACHE = {}


def _get_program():
    if "nc" not in _PROGRAM_CACHE:
        _PROGRAM_CACHE["nc"] = _build_program()
    return _PROGRAM_CACHE["nc"]


def _host_prep(x, idx, valid, geo_bias, Wq, Wk, Wv, Wo, bo):
    x2 = np.ascontiguousarray(np.asarray(x, dtype=np.float32).reshape(S, H))
    idx = np.asarray(idx).astype(np.int64)
    valid = np.asarray(valid).astype(bool)
    geo = np.asarray(geo_bias, dtype=np.float32)
    Wq = np.asarray(Wq, dtype=np.float32)
    Wk = np.asarray(Wk, dtype=np.float32)
    Wv = np.asarray(Wv, dtype=np.float32)
    Wo = np.asarray(Wo, dtype=np.float32)
    bo = np.asarray(bo, dtype=np.float32)

    qpos = np.arange(S, dtype=np.int64)[:, None]
    keep = valid & (idx <= qpos) & (idx >= 0)
    s_flat = idx[keep]
    q_flat = np.broadcast_to(qpos, idx.shape)[keep]
    lin = s_flat * S + q_flat

    bo_rep = np.ascontiguousarray(np.broadcast_to(bo[None, :], (128, H)))

    # den row order in phase 3 is r = l*8 + ci for head h = 2*ci + l
    sel16 = np.zeros((NH, H), dtype=np.float32)
    ch = np.arange(H)
    sel16[((ch // HD) % 2) * 8 + ch // 128, ch] = 1.0

    wq_scaled = Wq / np.sqrt(HD)
    xt_bf = np.ascontiguousarray(x2.T).astype(ml_dtypes.bfloat16)

    in_maps = []
    for core in range(NC):
        e_pack = np.empty((N_TILES, SC, HPC, QT), dtype=np.float16)
        for l in range(HPC):
            h = HPC * core + l
            w = np.exp(geo[h][keep].astype(np.float64))
            eT = np.bincount(lin, weights=w, minlength=S * S).reshape(S, S)
            for n, (t, c) in enumerate(TILE_LIST):
                e_pack[n, :, l, :] = eT[
                    c * SC : (c + 1) * SC, t * QT : (t + 1) * QT
                ].astype(np.float16)
        cs = slice(128 * core, 128 * (core + 1))
        in_maps.append(
            {
                "xt": xt_bf,
                "wq": np.ascontiguousarray(wq_scaled[:, cs]).astype(ml_dtypes.bfloat16),
                "wk": np.ascontiguousarray(Wk[:, cs]).astype(ml_dtypes.bfloat16),
                "wv": np.ascontiguousarray(Wv[:, cs]).astype(ml_dtypes.bfloat16),
                "wo": Wo.astype(ml_dtypes.bfloat16),
                "bo_rep": bo_rep,
                "e_pack": e_pack,
                "sel16": sel16,
            }
        )
    return in_maps


def kernel(x, idx, valid, geo_bias, Wq, Wk, Wv, Wo, bo):
    b, s, h = np.asarray(x).shape
    assert (b, s, h) == (1, S, H)
    in_maps = _host_prep(x, idx, valid, geo_bias, Wq, Wk, Wv, Wo, bo)
    nc = _get_program()
    res = run_bass_kernel_spmd(nc, in_maps, core_ids=list(range(NC)))
    # core c owns query rows t*512 + c*64 + j at its local column t*64 + j
    parts = np.stack([res.results[c]["y_part"] for c in range(NC)])  # [c, 512, H]
    y = parts.reshape(NC, NQT, 64, H).transpose(1, 0, 2, 3).reshape(S, H)
    return y.reshape(1, S, H).astype(np.float32)
